# revision 11
# baseline (speedup 1.0000x reference)
"""TRN2 Bass kernel for 16-head causal MHA (B=4, T=2048, C=2048), fp32.

Sharding: 8 cores = 4 batches x 2 head-groups (8 heads each).  Each core
computes q/k/v projections for its head group on its batch (tensor-parallel
column split of Wq/Wk/Wv), causal flash-style attention in the S^T layout
(scores computed transposed so softmax normalization is a partition-dim
reduction done with a ones-matmul), and a partial output projection with the
row slice of Wp.  The two head-group partials per batch are summed on the
host (the "all-reduce after proj" step), plus the output bias.

All matmuls run in float32r (full PE rate at N>=256, fp32-equivalent accuracy
as measured on HW).  Softmax uses exp without max-subtraction (scores are
O(+-10) for this problem's 0.02-scaled weights; exp is computed in fp32 from
the fp32 PSUM scores, so there is no overflow risk), with the causal mask
applied additively (-1e10) on diagonal blocks before the exp, and strictly
above-diagonal blocks skipped entirely.
"""
import math
import os

import numpy as np

import concourse.bass as bass
import concourse.tile as tile
from concourse import bacc, mybir
from concourse.bass_utils import run_bass_kernel_spmd

f32 = mybir.dt.float32
f32r = mybir.dt.float32r
AF = mybir.ActivationFunctionType

N_CORES = 8
HD = 128                      # head dim
NEG = -1e10                   # additive causal mask value

# results of the last run_bass_kernel_spmd call (for test harness profiling)
LAST_RESULT = None


def build_nc(T=2048, E=2048, D=1024, NOD=2048, TG=512, bias=False, num_devices=N_CORES,
             phases=("ab", "c", "d"), cparts=("mask", "dsum", "scat", "bcast", "mul")):
    """Build + compile the per-core Bass program.

    T: sequence length; E: embedding (contraction) dim; D: this core's head
    slice width (NH = D/128 heads); NOD: output projection width; TG: q-group
    width for attention; bias: if True, inputs carry one extra 128-row chunk
    holding [bias; zeros] against an xT with a ones row.
    """
    NH = D // HD
    EC = E // 128 + (1 if bias else 0)
    Ep = EC * 128
    TC = T // 128            # 128-row tiles along T
    TGC = T // TG            # q groups
    NMASK = TG // 128        # diagonal mask variants
    ODG = NOD // 512
    scale = 1.0 / math.sqrt(HD)
    VDG = 256                # v-phase d-group width

    nc = bacc.Bacc("TRN2", target_bir_lowering=False, debug=False,
                   num_devices=num_devices)

    xT = nc.dram_tensor("xT", [Ep, T], f32r, kind="ExternalInput")
    wq = nc.dram_tensor("wq", [Ep, D], f32r, kind="ExternalInput")
    wk = nc.dram_tensor("wk", [Ep, D], f32r, kind="ExternalInput")
    wv = nc.dram_tensor("wv", [Ep, D], f32r, kind="ExternalInput")
    wp = nc.dram_tensor("wp", [D, NOD], f32r, kind="ExternalInput")
    ones_d = nc.dram_tensor("ones", [128, 1], f32r, kind="ExternalInput")
    masks_d = nc.dram_tensor("masks", [NMASK, 128, TG], f32, kind="ExternalInput")
    y_d = nc.dram_tensor("y", [T, NOD], f32, kind="ExternalOutput")

    qt_sp = nc.dram_tensor("qt_sp", [D, T], f32r, kind="Internal")
    kt_sp = nc.dram_tensor("kt_sp", [D, T], f32r, kind="Internal")
    v_sp = nc.dram_tensor("v_sp", [T, D], f32r, kind="Internal")
    dscr = nc.dram_tensor("dscr", [NH * TGC, TG], f32, kind="Internal")
    dscr_ap = dscr.ap()
    dscr2 = nc.dram_tensor("dscr2", [NH * TGC, TG], f32, kind="Internal")
    dscr2_ap = dscr2.ap()

    with tile.TileContext(nc) as tc:
        # ---------------- phase A+B: q/k/v projections ----------------
        with (
            tc.tile_pool(name="xt", bufs=1) as xt_pool,
            tc.tile_pool(name="wstream", bufs=2) as w_pool,
            tc.tile_pool(name="ab_stage", bufs=4) as ab_stage,
            tc.tile_pool(name="ab_psum", bufs=8, space="PSUM") as ab_psum,
        ):
            xt_sb = xt_pool.tile([128, EC * T], f32r)
            for e in range(EC):
                nc.sync.dma_start(
                    xt_sb[:, e * T:(e + 1) * T],
                    xT[e * 128:(e + 1) * 128, :],
                )

            def xt_e(e):
                return xt_sb[:, e * T:(e + 1) * T]

            # Q^T and K^T in [d, t] layout: lhsT = W column chunk, rhs = xT.
            # Groups run in e-major waves of 8 (one PSUM bank each) so the PE
            # has 8 matmuls ready per arriving xT chunk during the input load.
            groups = [(w_i, dc, tg)
                      for w_i in range(2)
                      for dc in range(D // 128)
                      for tg in range(T // TG)]
            spills = (qt_sp, kt_sp)
            wds = (wq, wk)
            wcols = {}
            for wstart in range(0, len(groups), 8):
                wave = groups[wstart:wstart + 8]
                for w_i, dc, tg in wave:
                    if (w_i, dc) not in wcols:
                        wcol = w_pool.tile([128, EC * 128], f32r, tag="wcol")
                        nc.sync.dma_start(
                            wcol.rearrange("p (ec d) -> p ec d", ec=EC),
                            wds[w_i].rearrange("(ec p) d -> p ec d", p=128)[
                                :, :, dc * 128:(dc + 1) * 128],
                        )
                        wcols = {(w_i, dc): wcol, **{k: v for k, v in wcols.items()
                                                     if k[1] >= dc - 1}}
                pss = {}
                for w_i, dc, tg in wave:
                    pss[(w_i, dc, tg)] = ab_psum.tile(
                        [128, TG], f32, tag="abps", name=f"abps_{w_i}_{dc}_{tg}")
                for e in range(EC):
                    for w_i, dc, tg in wave:
                        nc.tensor.matmul(
                            pss[(w_i, dc, tg)][:],
                            wcols[(w_i, dc)][:, e * 128:(e + 1) * 128],
                            xt_e(e)[:, tg * TG:(tg + 1) * TG],
                            start=(e == 0), stop=(e == EC - 1),
                        )
                for w_i, dc, tg in wave:
                    st = ab_stage.tile([128, TG], f32r, tag="abst")
                    nc.scalar.copy(st[:], pss[(w_i, dc, tg)][:])
                    nc.sync.dma_start(
                        spills[w_i][dc * 128:(dc + 1) * 128, tg * TG:(tg + 1) * TG],
                        st[:],
                    )

            # V in natural [t, d] layout: lhsT = xT chunk, rhs = Wv chunk
            for dg in range(D // VDG):
                wvg = w_pool.tile([128, EC * VDG], f32r, tag="wvg")
                nc.sync.dma_start(
                    wvg.rearrange("p (ec d) -> p ec d", ec=EC),
                    wv.rearrange("(ec p) d -> p ec d", p=128)[
                        :, :, dg * VDG:(dg + 1) * VDG],
                )
                for tt in range(TC):
                    ps = ab_psum.tile([128, VDG], f32, tag="abps")
                    for e in range(EC):
                        nc.tensor.matmul(
                            ps[:],
                            xt_e(e)[:, tt * 128:(tt + 1) * 128],
                            wvg[:, e * VDG:(e + 1) * VDG],
                            start=(e == 0), stop=(e == EC - 1),
                        )
                    st = ab_stage.tile([128, VDG], f32r, tag="abst")
                    nc.scalar.copy(st[:], ps[:])
                    nc.sync.dma_start(
                        v_sp[tt * 128:(tt + 1) * 128, dg * VDG:(dg + 1) * VDG],
                        st[:],
                    )

        # ---------------- phase C: attention ----------------
        with tc.tile_pool(name="atn", bufs=1) as atn_pool:
            atn_all = atn_pool.tile([128, NH * T], f32r)
            with (
                tc.tile_pool(name="heads", bufs=2) as h_pool,
                tc.tile_pool(name="cwork", bufs=4) as c_pool,
                tc.tile_pool(name="consts", bufs=1) as const_pool,
                tc.tile_pool(name="c_psum_s", bufs=3, space="PSUM") as c_psum_s,
                tc.tile_pool(name="c_psum_a", bufs=3, space="PSUM") as c_psum_a,
                tc.tile_pool(name="c_psum_d", bufs=2, space="PSUM") as c_psum_d,
            ):
                ones_sb = const_pool.tile([128, 1], f32r)
                nc.sync.dma_start(ones_sb[:], ones_d[:])
                masks_sb = const_pool.tile([128, NMASK * TG], f32)
                nc.sync.dma_start(
                    masks_sb.rearrange("p (j q) -> p j q", j=NMASK),
                    masks_d.rearrange("j p q -> p j q"),
                )

                pending_norm = []

                def emit_norm(h, qg, slot, atn_u, dcol):
                    rcol = c_pool.tile([128, TG // 128], f32, tag="rcol",
                                       name=f"rcol_{slot}")
                    nc.vector.reciprocal(rcol[:], dcol[:])
                    nc.sync.dma_start(
                        bass.AP(tensor=dscr_ap.tensor, offset=slot * TG,
                                ap=[[1, 128], [128, TG // 128]]),
                        rcol[:],
                    )
                    recipB = c_pool.tile([128, TG], f32, tag="recipB",
                                         name=f"recipB_{slot}")
                    nc.gpsimd.dma_start(
                        out=recipB[:],
                        in_=bass.AP(tensor=dscr_ap.tensor, offset=slot * TG,
                                    ap=[[0, 128], [1, TG]]),
                    )
                    nc.gpsimd.tensor_mul(
                        atn_all[:, h * T + qg * TG:h * T + (qg + 1) * TG],
                        atn_u[:], recipB[:])

                for h in range(NH if "c" in phases else 0):
                    qt_h = h_pool.tile([128, T], f32r, tag="qt")
                    nc.sync.dma_start(qt_h[:], qt_sp[h * 128:(h + 1) * 128, :])
                    kt_h = h_pool.tile([128, T], f32r, tag="kt")
                    nc.sync.dma_start(kt_h[:], kt_sp[h * 128:(h + 1) * 128, :])
                    v_h = h_pool.tile([128, T], f32r, tag="vh")
                    nc.sync.dma_start(
                        v_h.rearrange("p (tc d) -> p tc d", d=128),
                        v_sp.rearrange("(tc p) d -> p tc d", p=128)[
                            :, :, h * 128:(h + 1) * 128],
                    )
                    for qg in range(TGC):
                        nk = (qg + 1) * NMASK
                        atn_ps = c_psum_a.tile([128, TG], f32, tag="atnps")
                        dsum_ps = c_psum_d.tile([1, TG], f32, tag="dsum")
                        for kc in range(nk):
                            s_ps = c_psum_s.tile([128, TG], f32, tag="sps")
                            nc.tensor.matmul(
                                s_ps[:],
                                kt_h[:, kc * 128:(kc + 1) * 128],
                                qt_h[:, qg * TG:(qg + 1) * TG],
                                start=True, stop=True,
                            )
                            j = kc - qg * NMASK
                            if j >= 0 and "mask" in cparts:
                                nc.vector.tensor_add(
                                    s_ps[:], s_ps[:],
                                    masks_sb[:, j * TG:(j + 1) * TG])
                            p_t = c_pool.tile([128, TG], f32r, tag="pt")
                            nc.scalar.activation(p_t[:], s_ps[:], AF.Exp, scale=scale)
                            nc.tensor.matmul(
                                atn_ps[:],
                                v_h[:, kc * 128:(kc + 1) * 128],
                                p_t[:],
                                start=(kc == 0), stop=(kc == nk - 1),
                            )
                            nc.tensor.matmul(
                                dsum_ps[:], ones_sb[:], p_t[:],
                                start=(kc == 0), stop=(kc == nk - 1),
                            )
                        # denominator: partition-sum via ones-matmul, then
                        # reciprocal row scattered/broadcast via DRAM roundtrip
                        slot = h * TGC + qg
                        # stage 1 of the normalization: copy the attn psum
                        # and denominator row out, scatter the row to a
                        # [128, TG//128] column layout via a DRAM hop.
                        atn_u = c_pool.tile([128, TG], f32, tag="atnu")
                        nc.vector.tensor_copy(atn_u[:], atn_ps[:])
                        dsum_sb = c_pool.tile([1, TG], f32, tag="dsum_sb")
                        nc.vector.tensor_copy(dsum_sb[:], dsum_ps[:])
                        nc.sync.dma_start(dscr2[slot:slot + 1, :], dsum_sb[:])
                        dcol = c_pool.tile([128, TG // 128], f32, tag="dcol")
                        nc.sync.dma_start(
                            dcol[:],
                            bass.AP(tensor=dscr2_ap.tensor, offset=slot * TG,
                                    ap=[[1, 128], [128, TG // 128]]),
                        )
                        # stage 2 (recip on all 128 lanes + broadcast +
                        # multiply) is deferred one group so the DVE queue
                        # never waits on the DMA hop latency.
                        pending_norm.append((h, qg, slot, atn_u, dcol))
                        if len(pending_norm) > 1:
                            emit_norm(*pending_norm.pop(0))

                for args in pending_norm:
                    emit_norm(*args)
                pending_norm.clear()

            # ---------------- phase D: output projection ----------------
            with (
                tc.tile_pool(name="dwork", bufs=2) as d_pool,
                tc.tile_pool(name="d_stage", bufs=4) as d_stage,
                tc.tile_pool(name="d_psum", bufs=3, space="PSUM") as d_psum,
            ):
                for og in range(ODG if "d" in phases else 0):
                    wpog = d_pool.tile([128, NH * 512], f32r, tag="wpog")
                    nc.sync.dma_start(
                        wpog.rearrange("p (dc o) -> p dc o", dc=NH),
                        wp.rearrange("(dc p) o -> p dc o", p=128)[
                            :, :, og * 512:(og + 1) * 512],
                    )
                    for tt in range(TC):
                        ps = d_psum.tile([128, 512], f32, tag="yps")
                        for hc in range(NH):
                            nc.tensor.matmul(
                                ps[:],
                                atn_all[:, hc * T + tt * 128:hc * T + (tt + 1) * 128],
                                wpog[:, hc * 512:(hc + 1) * 512],
                                start=(hc == 0), stop=(hc == NH - 1),
                            )
                        st = d_stage.tile([128, 512], f32, tag="yst")
                        nc.scalar.copy(st[:], ps[:])
                        nc.sync.dma_start(
                            y_d[tt * 128:(tt + 1) * 128, og * 512:(og + 1) * 512],
                            st[:],
                        )

    nc.compile()
    return nc


def _make_masks(TG):
    """masks[j][kk, qq] = 0 where kk <= qq - 128*j else NEG."""
    NMASK = TG // 128
    kk = np.arange(128)[:, None]
    qq = np.arange(TG)[None, :]
    return np.stack(
        [np.where(kk <= qq - 128 * j, 0.0, NEG) for j in range(NMASK)]
    ).astype(np.float32)


def _augment(mat_t, bias_row, pad_to):
    """Append [bias_row; zeros] below mat_t so it has pad_to rows."""
    extra = np.zeros((pad_to - mat_t.shape[0], mat_t.shape[1]), np.float32)
    extra[0] = bias_row
    return np.concatenate([mat_t, extra], axis=0)


_NC_CACHE = {}


def _get_nc(bias):
    key = bias
    if key not in _NC_CACHE:
        _NC_CACHE[key] = build_nc(bias=bias)
    return _NC_CACHE[key]


def kernel(x, Wq, bq, Wk, bk, Wv, bv, Wp, bp):
    global LAST_RESULT
    x = np.ascontiguousarray(np.asarray(x, np.float32))
    Wq, bq = np.asarray(Wq, np.float32), np.asarray(bq, np.float32)
    Wk, bk = np.asarray(Wk, np.float32), np.asarray(bk, np.float32)
    Wv, bv = np.asarray(Wv, np.float32), np.asarray(bv, np.float32)
    Wp, bp = np.asarray(Wp, np.float32), np.asarray(bp, np.float32)

    B, T, C = x.shape
    assert (B, T, C) == (4, 2048, 2048), (B, T, C)
    D = 1024  # head-group width: 8 heads per core
    bias = bool(np.any(bq) or np.any(bk) or np.any(bv))
    nc = _get_nc(bias)

    masks = _make_masks(512)
    ones = np.ones((128, 1), np.float32)
    Ep = C + 128 if bias else C

    in_maps = []
    for c in range(N_CORES):
        b, g = c // 2, c % 2
        xt = x[b].T
        wq_g = Wq[:, g * D:(g + 1) * D]
        wk_g = Wk[:, g * D:(g + 1) * D]
        wv_g = Wv[:, g * D:(g + 1) * D]
        if bias:
            xt = _augment(xt, np.ones(T, np.float32), Ep)
            wq_g = _augment(wq_g, bq[g * D:(g + 1) * D], Ep)
            wk_g = _augment(wk_g, bk[g * D:(g + 1) * D], Ep)
            wv_g = _augment(wv_g, bv[g * D:(g + 1) * D], Ep)
        in_maps.append({
            "xT": np.ascontiguousarray(xt),
            "wq": np.ascontiguousarray(wq_g),
            "wk": np.ascontiguousarray(wk_g),
            "wv": np.ascontiguousarray(wv_g),
            "wp": np.ascontiguousarray(Wp[g * D:(g + 1) * D, :]),
            "ones": ones,
            "masks": masks,
        })

    trace = bool(os.environ.get("MHA_TRACE"))
    res = run_bass_kernel_spmd(nc, in_maps, core_ids=list(range(N_CORES)),
                               trace=trace)
    LAST_RESULT = res

    out = np.empty((B, T, C), np.float32)
    for b in range(B):
        out[b] = res.results[2 * b]["y"] + res.results[2 * b + 1]["y"]
    out += bp[None, None, :]
    return out
